# revision 12
# baseline (speedup 1.0000x reference)
"""BiMamba block Trainium2 kernel.

Sharding: 8 cores = (direction f/b) x (batch 0/1) x (d_inner half 0/1).
Each core is fully independent (no collectives): it computes LN(x[b]),
in_proj u (full 2048, needed for x_proj) + its z half, depthwise causal
conv via PE diag-matmuls, silu, x_proj -> (dt|B|C), dt half, the
selective scan over its 1024 channels (d on partitions, 16 state
segments x time in the free dim, one tensor_tensor_scan per 8-state
slab), gating, and its out_proj column block.  Host flips the sequence
for backward cores and sums the 8 partial outputs + residual.
"""

import sys

sys.path.insert(0, "/opt/trn_rl_repo")

import numpy as np

import concourse.bass as bass
import concourse.mybir as mybir
from concourse import bacc
from concourse.tile import TileContext
from concourse.bass_utils import run_bass_kernel_spmd

FP32 = mybir.dt.float32
BF16 = mybir.dt.bfloat16
AX = mybir.AxisListType
OP = mybir.AluOpType
AF = mybir.ActivationFunctionType

P = 128
L = 1024          # sequence length
DM = 1024         # d_model
DI = 2048         # d_inner
DH = 1024         # d_inner half per core
DSTATE = 16
DTRANK = 64
DCONV = 4
NT = L // P       # 8 t-tiles
NKM = DM // P     # 8 d_model tiles
NMU = DI // P     # 16 u M-tiles
NMH = DH // P     # 8 half M-tiles
GSEG = 8          # states per scan slab
SLABF = GSEG * L  # slab free size


def build_program(finalize=True):
    nc = bacc.Bacc("TRN2", target_bir_lowering=False, debug=False)

    # ---- DRAM I/O (per-core shards; same names on every core) ----
    xin = nc.dram_tensor("xin", (L, DM), FP32, kind="ExternalInput")
    wuT = nc.dram_tensor("wuT", (DM, DI), BF16, kind="ExternalInput")
    wzT = nc.dram_tensor("wzT", (DM, DH), BF16, kind="ExternalInput")
    bu = nc.dram_tensor("bu", (P, NMU), FP32, kind="ExternalInput")
    bz = nc.dram_tensor("bz", (P, NMH), FP32, kind="ExternalInput")
    convd = nc.dram_tensor("convd", (DCONV, NMU, P, P), BF16, kind="ExternalInput")
    convb = nc.dram_tensor("convb", (P, NMU), FP32, kind="ExternalInput")
    wxpT = nc.dram_tensor("wxpT", (DI, 96), BF16, kind="ExternalInput")
    wdtT = nc.dram_tensor("wdtT", (DTRANK, DH), BF16, kind="ExternalInput")
    dtb = nc.dram_tensor("dtb", (P, NMH), FP32, kind="ExternalInput")
    Amat = nc.dram_tensor("Amat", (P, P), FP32, kind="ExternalInput")
    Dvec = nc.dram_tensor("Dvec", (P, NMH), FP32, kind="ExternalInput")
    woT = nc.dram_tensor("woT", (DH, DM), BF16, kind="ExternalInput")
    ident = nc.dram_tensor("ident", (P, P), BF16, kind="ExternalInput")
    outp = nc.dram_tensor("outp", (DM, L), FP32, kind="ExternalOutput")

    with TileContext(nc) as tc:
        const = tc.alloc_tile_pool(name="const", bufs=1)
        main = tc.alloc_tile_pool(name="main", bufs=1)

        bu_t = const.tile((P, NMU), FP32, name="bu_t")
        bz_t = const.tile((P, NMH), FP32, name="bz_t")
        convb_t = const.tile((P, NMU), FP32, name="convb_t")
        dtb_t = const.tile((P, NMH), FP32, name="dtb_t")
        A_t = const.tile((P, P), FP32, name="A_t")
        D_t = const.tile((P, NMH), FP32, name="D_t")
        id_t = const.tile((P, P), BF16, name="id_t")
        for dst, src in ((bu_t, bu), (bz_t, bz), (convb_t, convb),
                         (dtb_t, dtb), (A_t, Amat), (D_t, Dvec), (id_t, ident)):
            nc.sync.dma_start(out=dst[:], in_=src[:])

        # persistent activations
        u_silu = [main.tile((P, L), BF16, name=f"usl{m}", tag=f"usl{m}") for m in range(NMU)]
        z_silu = [main.tile((P, L), BF16, name=f"zsl{m}", tag=f"zsl{m}") for m in range(NMH)]
        dt_sb = [main.tile((P, L), BF16, name=f"dt{m}", tag=f"dt{m}") for m in range(NMH)]
        ygate = [main.tile((P, L), BF16, name=f"yg{m}", tag=f"yg{m}") for m in range(NMH)]
        dbc_bf = main.tile((96, L), BF16, name="dbc_bf", tag="dbc")

        # ---------- front phase: LN -> transpose -> in_proj -> conv ----------
        front = tc.alloc_tile_pool(name="front", bufs=1)
        wpool = tc.alloc_tile_pool(name="wpool", bufs=16)
        ppool = tc.alloc_tile_pool(name="ppool", bufs=3, space="PSUM")
        trpool = tc.alloc_tile_pool(name="trpool", bufs=2, space="PSUM")

        xnT = [front.tile((P, L), BF16, name=f"xnT{k}", tag=f"xnT{k}") for k in range(NKM)]

        for tt in range(NT):
            xt = front.tile((P, DM), FP32, name="xt", tag="xt", bufs=2)
            nc.sync.dma_start(out=xt[:], in_=xin[tt * P:(tt + 1) * P, :])
            ssum = front.tile((P, 1), FP32, name="ssum", tag="stats", bufs=4)
            xsq = front.tile((P, DM), FP32, name="xsq", tag="xsq", bufs=2)
            nc.scalar.activation(xsq[:], xt[:], AF.Identity, accum_out=ssum[:])
            ssq = front.tile((P, 1), FP32, name="ssq", tag="stats", bufs=4)
            nc.scalar.activation(xsq[:], xt[:], AF.Square, accum_out=ssq[:])
            mu = front.tile((P, 1), FP32, name="mu", tag="stats", bufs=4)
            nc.vector.tensor_scalar_mul(mu[:], ssum[:], 1.0 / DM)
            ex2 = front.tile((P, 1), FP32, name="ex2", tag="stats", bufs=4)
            nc.vector.tensor_scalar_mul(ex2[:], ssq[:], 1.0 / DM)
            mu2 = front.tile((P, 1), FP32, name="mu2", tag="stats", bufs=4)
            nc.vector.tensor_mul(mu2[:], mu[:], mu[:])
            var = front.tile((P, 1), FP32, name="var", tag="stats", bufs=4)
            nc.vector.tensor_tensor(var[:], ex2[:], mu2[:], OP.subtract)
            nc.vector.tensor_scalar_add(var[:], var[:], 1e-5)
            sd = front.tile((P, 1), FP32, name="sd", tag="stats", bufs=4)
            nc.scalar.activation(sd[:], var[:], AF.Sqrt)
            r = front.tile((P, 1), FP32, name="r", tag="stats", bufs=4)
            nc.vector.reciprocal(r[:], sd[:])
            xn = front.tile((P, DM), BF16, name="xn", tag="xn", bufs=2)
            nc.vector.tensor_scalar(xn[:], xt[:], mu[:], r[:], OP.subtract, OP.mult)
            # transpose this t-tile into the 8 xnT tiles
            for kk in range(NKM):
                tr = trpool.tile((P, P), BF16, name="tr", tag="tr")
                nc.tensor.transpose(tr[:], xn[:, kk * P:(kk + 1) * P], id_t[:])
                nc.scalar.copy(xnT[kk][:, tt * P:(tt + 1) * P], tr[:])

        # in_proj: u (16 M-tiles) then z (8 M-tiles)
        u_pre = []
        for m in range(NMU):
            ps = ppool.tile((P, L), FP32, name="ps", tag="mm")
            for k in range(NKM):
                wt = wpool.tile((P, P), BF16, name="wt", tag="wt")
                nc.sync.dma_start(out=wt[:], in_=wuT[k * P:(k + 1) * P, m * P:(m + 1) * P])
                for c in range(2):
                    nc.tensor.matmul(ps[:, c * 512:(c + 1) * 512], wt[:],
                                     xnT[k][:, c * 512:(c + 1) * 512],
                                     start=(k == 0), stop=(k == NKM - 1))
            up = front.tile((P, L + DCONV), BF16, name="up", tag="upre", bufs=3)
            nc.vector.memset(up[:, 0:DCONV], 0.0)
            nc.scalar.activation(up[:, DCONV:], ps[:], AF.Identity, bias=bu_t[:, m:m + 1])
            u_pre.append(up)
            # conv for this m-tile: 4 shifted diag matmuls
            pc = ppool.tile((P, L), FP32, name="pc", tag="mm")
            for k in range(DCONV):
                cw = wpool.tile((P, P), BF16, name="cw", tag="wt")
                nc.sync.dma_start(out=cw[:], in_=convd[k, m])
                for c in range(2):
                    nc.tensor.matmul(pc[:, c * 512:(c + 1) * 512], cw[:],
                                     up[:, k + 1 + c * 512:k + 1 + (c + 1) * 512],
                                     start=(k == 0), stop=(k == DCONV - 1))
            sig = front.tile((P, L), BF16, name="sig", tag="sig", bufs=3)
            nc.scalar.activation(sig[:], pc[:], AF.Sigmoid, bias=convb_t[:, m:m + 1])
            ucb = front.tile((P, L), BF16, name="ucb", tag="ucb", bufs=3)
            nc.scalar.activation(ucb[:], pc[:], AF.Identity, bias=convb_t[:, m:m + 1])
            nc.vector.tensor_mul(u_silu[m][:], sig[:], ucb[:])

        for m in range(NMH):
            ps = ppool.tile((P, L), FP32, name="ps", tag="mm")
            for k in range(NKM):
                wt = wpool.tile((P, P), BF16, name="wt", tag="wt")
                nc.sync.dma_start(out=wt[:], in_=wzT[k * P:(k + 1) * P, m * P:(m + 1) * P])
                for c in range(2):
                    nc.tensor.matmul(ps[:, c * 512:(c + 1) * 512], wt[:],
                                     xnT[k][:, c * 512:(c + 1) * 512],
                                     start=(k == 0), stop=(k == NKM - 1))
            sigz = front.tile((P, L), BF16, name="sigz", tag="sig", bufs=3)
            nc.scalar.activation(sigz[:], ps[:], AF.Sigmoid, bias=bz_t[:, m:m + 1])
            zcb = front.tile((P, L), BF16, name="zcb", tag="ucb", bufs=3)
            nc.scalar.activation(zcb[:], ps[:], AF.Identity, bias=bz_t[:, m:m + 1])
            nc.vector.tensor_mul(z_silu[m][:], sigz[:], zcb[:])

        # x_proj: dbc (96, L) = wxpT.T @ u_silu
        pxp = ppool.tile((96, L), FP32, name="pxp", tag="mm")
        for k in range(NMU):
            wt = wpool.tile((P, 96), BF16, name="wtx", tag="wtx", bufs=4)
            nc.sync.dma_start(out=wt[:], in_=wxpT[k * P:(k + 1) * P, :])
            for c in range(2):
                nc.tensor.matmul(pxp[:, c * 512:(c + 1) * 512], wt[:],
                                 u_silu[k][:, c * 512:(c + 1) * 512],
                                 start=(k == 0), stop=(k == NMU - 1))
        nc.scalar.copy(dbc_bf[:], pxp[:])

        # dt half: softplus(wdtT.T @ dbc[:64] + dtb)
        wdt = const.tile((DTRANK, DH), BF16, name="wdt")
        nc.sync.dma_start(out=wdt[:], in_=wdtT[:])
        for m in range(NMH):
            ps = ppool.tile((P, L), FP32, name="psd", tag="mm")
            for c in range(2):
                nc.tensor.matmul(ps[:, c * 512:(c + 1) * 512], wdt[:, m * P:(m + 1) * P],
                                 dbc_bf[0:DTRANK, c * 512:(c + 1) * 512],
                                 start=True, stop=True)
            edt = front.tile((P, L), FP32, name="edt", tag="edt", bufs=2)
            nc.scalar.activation(edt[:], ps[:], AF.Exp, bias=dtb_t[:, m:m + 1])
            nc.scalar.activation(dt_sb[m][:], edt[:], AF.Ln, bias=1.0)

        trpool.release()
        wpool.release()
        front.release()

        # ---------- scan phase ----------
        scanp = tc.alloc_tile_pool(name="scanp", bufs=4)
        bcp = tc.alloc_tile_pool(name="bcp", bufs=2)
        half = 0  # u_silu index offset is chosen on host via weight sharding
        # NOTE: the dt/z/out shards shipped by the host correspond to
        # u_silu tiles [hoff .. hoff+8); hoff is encoded by shipping the
        # matching halves of conv/dt/out weights, with u channel tiles
        # relabeled so that tiles 0..7 of the half come first.  The host
        # arranges wuT/convd so that THIS core's half occupies m=0..7.
        ones_row = const.tile((1, P), BF16, name="ones_row")
        nc.vector.memset(ones_row[:], 1.0)
        for g in range(2):
            Bsl = bcp.tile((P, SLABF), BF16, name="Bsl", tag="bc")
            Csl = bcp.tile((P, SLABF), BF16, name="Csl", tag="bc")
            for j in range(GSEG):
                n = g * GSEG + j
                for dst, row in ((Bsl, DTRANK + n), (Csl, DTRANK + DSTATE + n)):
                    stage = scanp.tile((1, L), BF16, name="stage", tag="stage", bufs=4)
                    nc.sync.dma_start(out=stage[:], in_=dbc_bf[row:row + 1, :])
                    pb = ppool.tile((P, L), FP32, name="pb", tag="mm")
                    for c in range(2):
                        nc.tensor.matmul(pb[:, c * 512:(c + 1) * 512], ones_row[:],
                                         stage[:, c * 512:(c + 1) * 512],
                                         start=True, stop=True)
                    nc.scalar.copy(dst[:, j * L:(j + 1) * L], pb[:])
            for m in range(NMH):
                dtu = scanp.tile((P, L), BF16, name="dtu", tag="dtu", bufs=2)
                nc.vector.tensor_mul(dtu[:], dt_sb[m][:], u_silu[half + m][:])
                asl = scanp.tile((P, SLABF), BF16, name="asl", tag="slab")
                for j in range(GSEG):
                    n = g * GSEG + j
                    nc.scalar.activation(asl[:, j * L:(j + 1) * L], dt_sb[m][:],
                                         AF.Exp, scale=A_t[:, m * DSTATE + n:m * DSTATE + n + 1])
                nc.vector.memset(asl[:, 0:SLABF:L], 0.0)
                bsl = scanp.tile((P, SLABF), BF16, name="bsl", tag="slab")
                for j in range(GSEG):
                    nc.vector.tensor_mul(bsl[:, j * L:(j + 1) * L], dtu[:],
                                         Bsl[:, j * L:(j + 1) * L])
                hsl = scanp.tile((P, SLABF), BF16, name="hsl", tag="slab")
                nc.vector.tensor_tensor_scan(hsl[:], asl[:], bsl[:], 0.0, OP.mult, OP.add)
                csl = scanp.tile((P, SLABF), BF16, name="csl", tag="slab")
                nc.vector.tensor_mul(csl[:], hsl[:], Csl[:])
                # tree reduce the 8 segments -> (P, L), ping-pong via bsl
                nc.vector.tensor_add(bsl[:, 0:4 * L], csl[:, 0:4 * L], csl[:, 4 * L:8 * L])
                nc.vector.tensor_add(csl[:, 0:2 * L], bsl[:, 0:2 * L], bsl[:, 2 * L:4 * L])
                if g == 0:
                    nc.vector.tensor_add(ygate[m][:], csl[:, 0:L], csl[:, L:2 * L])
                else:
                    nc.vector.tensor_add(csl[:, 0:L], csl[:, 0:L], csl[:, L:2 * L])
                    # y = y_g0 + y_g1 + u*D ; then gate by silu(z)
                    nc.vector.tensor_add(ygate[m][:], ygate[m][:], csl[:, 0:L])
                    nc.vector.scalar_tensor_tensor(
                        ygate[m][:], u_silu[half + m][:], D_t[:, m:m + 1], ygate[m][:],
                        OP.mult, OP.add)
                    nc.vector.tensor_mul(ygate[m][:], ygate[m][:], z_silu[m][:])
        bcp.release()
        scanp.release()

        # ---------- out_proj ----------
        wop = tc.alloc_tile_pool(name="wop", bufs=4)
        opool = tc.alloc_tile_pool(name="opool", bufs=2)
        for m in range(NKM):
            ps = ppool.tile((P, L), FP32, name="pso", tag="mm")
            for k in range(NMH):
                wt = wop.tile((P, P), BF16, name="wto", tag="wto")
                nc.sync.dma_start(out=wt[:], in_=woT[k * P:(k + 1) * P, m * P:(m + 1) * P])
                for c in range(2):
                    nc.tensor.matmul(ps[:, c * 512:(c + 1) * 512], wt[:],
                                     ygate[k][:, c * 512:(c + 1) * 512],
                                     start=(k == 0), stop=(k == NMH - 1))
            osb = opool.tile((P, L), FP32, name="osb", tag="osb")
            nc.scalar.copy(osb[:], ps[:])
            nc.sync.dma_start(out=outp[m * P:(m + 1) * P, :], in_=osb[:])
        opool.release()
        wop.release()
        ppool.release()
        main.release()
        const.release()
    if finalize:
        nc.finalize()
    return nc


def _shards(inputs):
    """Build the 8 per-core input maps (numpy, fp32/bf16 via ml_dtypes)."""
    import ml_dtypes

    def bf(a):
        return np.asarray(a, np.float32).astype(ml_dtypes.bfloat16)

    x = np.asarray(inputs["x"], np.float32)
    g = np.asarray(inputs["ln_g"], np.float32)
    be = np.asarray(inputs["ln_b"], np.float32)
    ident = np.eye(P, dtype=np.float32)

    maps = []
    for d, pre in ((0, "f_"), (1, "b_")):
        in_w = np.asarray(inputs[pre + "in_w"], np.float32)
        conv_w = np.asarray(inputs[pre + "conv_w"], np.float32)
        conv_b = np.asarray(inputs[pre + "conv_b"], np.float32)
        xproj_w = np.asarray(inputs[pre + "xproj_w"], np.float32)
        dt_w = np.asarray(inputs[pre + "dt_w"], np.float32)
        dt_b = np.asarray(inputs[pre + "dt_b"], np.float32)
        Alog = np.asarray(inputs[pre + "Alog"], np.float32)
        Dv = np.asarray(inputs[pre + "D"], np.float32)
        out_w = np.asarray(inputs[pre + "out_w"], np.float32)
        A = -np.exp(Alog)  # (DI, DSTATE)

        for b in range(2):
            for h in range(2):
                sl = slice(h * DH, (h + 1) * DH)
                # u channel tiles reordered so this core's half comes first
                order = np.r_[h * DH:(h + 1) * DH, (1 - h) * DH:(2 - h) * DH] if h == 1 else np.arange(DI)
                w_u = in_w[:DI][order] * g[None, :]
                w_z = in_w[DI:][sl] * g[None, :]
                bu_full = (in_w[:DI][order] @ be)
                bz_full = (in_w[DI:][sl] @ be)
                cw = conv_w[order]
                cb = conv_b[order]
                convdiag = np.zeros((DCONV, NMU, P, P), np.float32)
                for k in range(DCONV):
                    for m in range(NMU):
                        np.fill_diagonal(convdiag[k, m], cw[m * P:(m + 1) * P, k])
                Ah = A[sl]  # (DH, 16)
                Amat = Ah.reshape(NMH, P, DSTATE).transpose(1, 0, 2).reshape(P, P)
                xp = xproj_w[:, order]  # (96, DI)
                xs = x[b] if d == 0 else x[b][::-1]
                m = {
                    "xin": np.ascontiguousarray(xs),
                    "wuT": bf(w_u.T),
                    "wzT": bf(w_z.T),
                    "bu": np.ascontiguousarray(bu_full.reshape(NMU, P).T),
                    "bz": np.ascontiguousarray(bz_full.reshape(NMH, P).T),
                    "convd": bf(convdiag),
                    "convb": np.ascontiguousarray(cb.reshape(NMU, P).T),
                    "wxpT": bf(xp.T),
                    "wdtT": bf(dt_w[sl].T),
                    "dtb": np.ascontiguousarray(dt_b[sl].reshape(NMH, P).T),
                    "Amat": np.ascontiguousarray(Amat),
                    "Dvec": np.ascontiguousarray(Dv[sl].reshape(NMH, P).T),
                    "woT": bf(out_w[:, sl].T),
                    "ident": bf(ident),
                }
                maps.append(m)
    return maps


_CACHE = {}


def kernel(**inputs):
    if "nc" not in _CACHE:
        _CACHE["nc"] = build_program()
    nc = _CACHE["nc"]
    maps = _shards(inputs)
    res = run_bass_kernel_spmd(nc, maps, list(range(8)))
    outs = res.results
    x = np.asarray(inputs["x"], np.float32)
    out = x.copy()
    i = 0
    for d in range(2):
        for b in range(2):
            for h in range(2):
                part = outs[i]["outp"].T  # (t, dmo)
                if d == 1:
                    part = part[::-1]
                out[b] += part
                i += 1
    return out


# revision 23
# speedup vs baseline: 1.1411x; 1.1411x over previous
"""BiMamba block Trainium2 kernel.

Sharding: 8 cores = (direction f/b) x (batch 0/1) x (d_inner half 0/1).
Each core is fully independent (no collectives): it computes LN(x[b]),
in_proj u (full 2048, needed for x_proj) + its z half, depthwise causal
conv via PE diag-matmuls, silu, x_proj -> (dt|B|C), dt half, the
selective scan over its 1024 channels (d on partitions, 16 state
segments x time in the free dim, one tensor_tensor_scan per 8-state
slab), gating, and its out_proj column block.  Host flips the sequence
for backward cores and sums the 8 partial outputs + residual.
"""

import sys

sys.path.insert(0, "/opt/trn_rl_repo")

import numpy as np

import concourse.bass as bass
import concourse.mybir as mybir
from concourse import bacc
from concourse.tile import TileContext
from concourse.bass_utils import run_bass_kernel_spmd

FP32 = mybir.dt.float32
BF16 = mybir.dt.bfloat16
AX = mybir.AxisListType
OP = mybir.AluOpType
AF = mybir.ActivationFunctionType

P = 128
L = 1024          # sequence length
DM = 1024         # d_model
DI = 2048         # d_inner
DH = 1024         # d_inner half per core
DSTATE = 16
DTRANK = 64
DCONV = 4
NT = L // P       # 8 t-tiles
NKM = DM // P     # 8 d_model tiles
NMU = DI // P     # 16 u M-tiles
NMH = DH // P     # 8 half M-tiles
GSEG = 8          # states per scan slab
SLABF = GSEG * L  # slab free size


def build_program(finalize=True):
    nc = bacc.Bacc("TRN2", target_bir_lowering=False, debug=False)

    # ---- DRAM I/O (per-core shards; same names on every core) ----
    xin = nc.dram_tensor("xin", (L, DM), FP32, kind="ExternalInput")
    wuT = nc.dram_tensor("wuT", (DM, DI), BF16, kind="ExternalInput")
    wzT = nc.dram_tensor("wzT", (DM, DH), BF16, kind="ExternalInput")
    bu = nc.dram_tensor("bu", (P, NMU), FP32, kind="ExternalInput")
    bz = nc.dram_tensor("bz", (P, NMH), FP32, kind="ExternalInput")
    convd = nc.dram_tensor("convd", (DCONV, NMU, P, P), BF16, kind="ExternalInput")
    convb = nc.dram_tensor("convb", (P, NMU), FP32, kind="ExternalInput")
    wxpT = nc.dram_tensor("wxpT", (DI, 96), BF16, kind="ExternalInput")
    wdtT = nc.dram_tensor("wdtT", (DTRANK, DH), BF16, kind="ExternalInput")
    dtb = nc.dram_tensor("dtb", (P, NMH), FP32, kind="ExternalInput")
    Amat = nc.dram_tensor("Amat", (P, P), FP32, kind="ExternalInput")
    Dvec = nc.dram_tensor("Dvec", (P, NMH), FP32, kind="ExternalInput")
    woT = nc.dram_tensor("woT", (DH, DM), BF16, kind="ExternalInput")
    ident = nc.dram_tensor("ident", (P, P), BF16, kind="ExternalInput")
    outp = nc.dram_tensor("outp", (DM, L), FP32, kind="ExternalOutput")

    with TileContext(nc) as tc:
        const = tc.alloc_tile_pool(name="const", bufs=1)
        main = tc.alloc_tile_pool(name="main", bufs=1)

        bu_t = const.tile((P, NMU), FP32, name="bu_t")
        bz_t = const.tile((P, NMH), FP32, name="bz_t")
        convb_t = const.tile((P, NMU), FP32, name="convb_t")
        dtb_t = const.tile((P, NMH), FP32, name="dtb_t")
        A_t = const.tile((P, P), FP32, name="A_t")
        D_t = const.tile((P, NMH), FP32, name="D_t")
        id_t = const.tile((P, P), BF16, name="id_t")
        for dst, src in ((bu_t, bu), (bz_t, bz), (convb_t, convb),
                         (dtb_t, dtb), (A_t, Amat), (D_t, Dvec), (id_t, ident)):
            nc.sync.dma_start(out=dst[:], in_=src[:])

        # persistent activations
        u_silu = [main.tile((P, L), BF16, name=f"usl{m}", tag=f"usl{m}") for m in range(NMU)]
        z_silu = [main.tile((P, L), BF16, name=f"zsl{m}", tag=f"zsl{m}") for m in range(NMH)]
        dt_sb = [main.tile((P, L), BF16, name=f"dt{m}", tag=f"dt{m}") for m in range(NMH)]
        ygate = [main.tile((P, L), BF16, name=f"yg{m}", tag=f"yg{m}") for m in range(NMH)]
        dbc_bf = main.tile((96, L), BF16, name="dbc_bf", tag="dbc")

        # ---------- front phase: LN -> transpose -> in_proj -> conv ----------
        front = tc.alloc_tile_pool(name="front", bufs=1)
        wpool = tc.alloc_tile_pool(name="wpool", bufs=16)
        ppool = tc.alloc_tile_pool(name="ppool", bufs=3, space="PSUM")
        trpool = tc.alloc_tile_pool(name="trpool", bufs=2, space="PSUM")

        xnT = [front.tile((P, L), BF16, name=f"xnT{k}", tag=f"xnT{k}") for k in range(NKM)]

        for tt in range(NT):
            xt = front.tile((P, DM), FP32, name="xt", tag="xt", bufs=2)
            nc.sync.dma_start(out=xt[:], in_=xin[tt * P:(tt + 1) * P, :])
            ssum = front.tile((P, 1), FP32, name="ssum", tag="stats", bufs=4)
            xsq = front.tile((P, DM), FP32, name="xsq", tag="xsq", bufs=1)
            nc.scalar.activation(xsq[:], xt[:], AF.Identity, accum_out=ssum[:])
            ssq = front.tile((P, 1), FP32, name="ssq", tag="stats", bufs=4)
            nc.scalar.activation(xsq[:], xt[:], AF.Square, accum_out=ssq[:])
            mu = front.tile((P, 1), FP32, name="mu", tag="stats", bufs=4)
            nc.vector.tensor_scalar_mul(mu[:], ssum[:], 1.0 / DM)
            ex2 = front.tile((P, 1), FP32, name="ex2", tag="stats", bufs=4)
            nc.vector.tensor_scalar_mul(ex2[:], ssq[:], 1.0 / DM)
            mu2 = front.tile((P, 1), FP32, name="mu2", tag="stats", bufs=4)
            nc.vector.tensor_mul(mu2[:], mu[:], mu[:])
            var = front.tile((P, 1), FP32, name="var", tag="stats", bufs=4)
            nc.vector.tensor_tensor(var[:], ex2[:], mu2[:], OP.subtract)
            nc.vector.tensor_scalar_add(var[:], var[:], 1e-5)
            sd = front.tile((P, 1), FP32, name="sd", tag="stats", bufs=4)
            nc.scalar.activation(sd[:], var[:], AF.Sqrt)
            r = front.tile((P, 1), FP32, name="r", tag="stats", bufs=4)
            nc.vector.reciprocal(r[:], sd[:])
            xn = front.tile((P, DM), BF16, name="xn", tag="xn", bufs=2)
            nc.vector.tensor_scalar(xn[:], xt[:], mu[:], r[:], OP.subtract, OP.mult)
            # transpose this t-tile into the 8 xnT tiles
            for kk in range(NKM):
                tr = trpool.tile((P, P), BF16, name="tr", tag="tr")
                nc.tensor.transpose(tr[:], xn[:, kk * P:(kk + 1) * P], id_t[:])
                nc.scalar.copy(xnT[kk][:, tt * P:(tt + 1) * P], tr[:])

        # in_proj: u (16 M-tiles) then z (8 M-tiles)
        u_pre = []
        for m in range(NMU):
            ps = ppool.tile((P, L), FP32, name="ps", tag="mm")
            for k in range(NKM):
                wt = wpool.tile((P, P), BF16, name="wt", tag="wt", bufs=8)
                nc.sync.dma_start(out=wt[:], in_=wuT[k * P:(k + 1) * P, m * P:(m + 1) * P])
                for c in range(2):
                    nc.tensor.matmul(ps[:, c * 512:(c + 1) * 512], wt[:],
                                     xnT[k][:, c * 512:(c + 1) * 512],
                                     start=(k == 0), stop=(k == NKM - 1))
            up = front.tile((P, L + DCONV), BF16, name="up", tag="upre", bufs=2)
            nc.vector.memset(up[:, 0:DCONV], 0.0)
            nc.scalar.activation(up[:, DCONV:], ps[:], AF.Identity, bias=bu_t[:, m:m + 1])
            u_pre.append(up)
            # conv for this m-tile: 4 shifted diag matmuls
            pc = ppool.tile((P, L), FP32, name="pc", tag="mm")
            for k in range(DCONV):
                cw = wpool.tile((P, P), BF16, name="cw", tag="cw", bufs=4)
                nc.sync.dma_start(out=cw[:], in_=convd[k, m])
                for c in range(2):
                    nc.tensor.matmul(pc[:, c * 512:(c + 1) * 512], cw[:],
                                     up[:, k + 1 + c * 512:k + 1 + (c + 1) * 512],
                                     start=(k == 0), stop=(k == DCONV - 1))
            sig = front.tile((P, L), BF16, name="sig", tag="sig", bufs=2)
            nc.scalar.activation(sig[:], pc[:], AF.Sigmoid, bias=convb_t[:, m:m + 1])
            ucb = front.tile((P, L), BF16, name="ucb", tag="ucb", bufs=2)
            nc.vector.tensor_scalar_add(ucb[:], pc[:], convb_t[:, m:m + 1])
            nc.vector.tensor_mul(u_silu[m][:], sig[:], ucb[:])

        # x_proj: dbc (96, L) = wxpT.T @ u_silu
        pxp = ppool.tile((96, L), FP32, name="pxp", tag="mm")
        for k in range(NMU):
            wtx = wpool.tile((P, 96), BF16, name="wtx", tag="wtx", bufs=4)
            nc.sync.dma_start(out=wtx[:], in_=wxpT[k * P:(k + 1) * P, :])
            for c in range(2):
                nc.tensor.matmul(pxp[:, c * 512:(c + 1) * 512], wtx[:],
                                 u_silu[k][:, c * 512:(c + 1) * 512],
                                 start=(k == 0), stop=(k == NMU - 1))
        nc.scalar.copy(dbc_bf[:], pxp[:])

        # dt half: softplus(wdtT.T @ dbc[:64] + dtb)
        wdt = const.tile((DTRANK, DH), BF16, name="wdt")
        nc.sync.dma_start(out=wdt[:], in_=wdtT[:])
        for m in range(NMH):
            ps = ppool.tile((P, L), FP32, name="psd", tag="mm")
            for c in range(2):
                nc.tensor.matmul(ps[:, c * 512:(c + 1) * 512], wdt[:, m * P:(m + 1) * P],
                                 dbc_bf[0:DTRANK, c * 512:(c + 1) * 512],
                                 start=True, stop=True)
            edt = front.tile((P, L), BF16, name="edt", tag="edt", bufs=2)
            nc.scalar.activation(edt[:], ps[:], AF.Exp, bias=dtb_t[:, m:m + 1])
            nc.scalar.activation(dt_sb[m][:], edt[:], AF.Ln, bias=1.0)

        for m in range(NMH):
            ps = ppool.tile((P, L), FP32, name="ps", tag="mm")
            for k in range(NKM):
                wt = wpool.tile((P, P), BF16, name="wt", tag="wt", bufs=8)
                nc.sync.dma_start(out=wt[:], in_=wzT[k * P:(k + 1) * P, m * P:(m + 1) * P])
                for c in range(2):
                    nc.tensor.matmul(ps[:, c * 512:(c + 1) * 512], wt[:],
                                     xnT[k][:, c * 512:(c + 1) * 512],
                                     start=(k == 0), stop=(k == NKM - 1))
            sigz = front.tile((P, L), BF16, name="sigz", tag="sig", bufs=2)
            nc.scalar.activation(sigz[:], ps[:], AF.Sigmoid, bias=bz_t[:, m:m + 1])
            zcb = front.tile((P, L), BF16, name="zcb", tag="ucb", bufs=2)
            nc.scalar.activation(zcb[:], ps[:], AF.Identity, bias=bz_t[:, m:m + 1])
            nc.vector.tensor_mul(z_silu[m][:], sigz[:], zcb[:])


        trpool.release()
        wpool.release()
        front.release()

        # ---------- scan phase ----------
        scanp = tc.alloc_tile_pool(name="scanp", bufs=5)
        bcp = tc.alloc_tile_pool(name="bcp", bufs=2)
        half = 0  # u_silu index offset is chosen on host via weight sharding
        # NOTE: the dt/z/out shards shipped by the host correspond to
        # u_silu tiles [hoff .. hoff+8); hoff is encoded by shipping the
        # matching halves of conv/dt/out weights, with u channel tiles
        # relabeled so that tiles 0..7 of the half come first.  The host
        # arranges wuT/convd so that THIS core's half occupies m=0..7.
        ones_row = const.tile((1, P), BF16, name="ones_row")
        nc.vector.memset(ones_row[:], 1.0)

        for g in range(2):
            Bsl = bcp.tile((P, SLABF), BF16, name="Bsl", tag="bc")
            Csl = bcp.tile((P, SLABF), BF16, name="Csl", tag="bc")
            for j in range(GSEG):
                n = g * GSEG + j
                for dst, row in ((Bsl, DTRANK + n), (Csl, DTRANK + DSTATE + n)):
                    stage = scanp.tile((1, L), BF16, name="stage", tag="stage", bufs=2)
                    nc.sync.dma_start(out=stage[:], in_=dbc_bf[row:row + 1, :])
                    pb = ppool.tile((P, L), FP32, name="pb", tag="mm")
                    for c in range(2):
                        nc.tensor.matmul(pb[:, c * 512:(c + 1) * 512], ones_row[:],
                                         stage[:, c * 512:(c + 1) * 512],
                                         start=True, stop=True)
                    nc.scalar.copy(dst[:, j * L:(j + 1) * L], pb[:])
            for m in range(NMH):
                dtu = scanp.tile((P, L), BF16, name="dtu", tag="dtu", bufs=2)
                nc.vector.tensor_mul(dtu[:], dt_sb[m][:], u_silu[half + m][:])
                asl = scanp.tile((P, SLABF), BF16, name="asl", tag="slab")
                for j in range(GSEG):
                    n = g * GSEG + j
                    nc.scalar.activation(asl[:, j * L:(j + 1) * L], dt_sb[m][:],
                                         AF.Exp, scale=A_t[:, m * DSTATE + n:m * DSTATE + n + 1])
                nc.vector.memset(asl[:, 0:SLABF:L], 0.0)
                bsl = scanp.tile((P, SLABF), BF16, name="bsl", tag="slab")
                for j in range(GSEG):
                    nc.vector.tensor_mul(bsl[:, j * L:(j + 1) * L], dtu[:],
                                         Bsl[:, j * L:(j + 1) * L])
                hsl = scanp.tile((P, SLABF), BF16, name="hsl", tag="slab")
                nc.vector.tensor_tensor_scan(hsl[:], asl[:], bsl[:], 0.0, OP.mult, OP.add)
                # ch = h*C into bsl (dead); tree reduce ping-pongs bsl/asl
                nc.vector.tensor_mul(bsl[:], hsl[:], Csl[:])
                nc.vector.tensor_add(asl[:, 0:4 * L], bsl[:, 0:4 * L], bsl[:, 4 * L:8 * L])
                nc.vector.tensor_add(bsl[:, 0:2 * L], asl[:, 0:2 * L], asl[:, 2 * L:4 * L])
                if g == 0:
                    nc.vector.tensor_add(ygate[m][:], bsl[:, 0:L], bsl[:, L:2 * L])
                else:
                    nc.vector.tensor_add(bsl[:, 0:L], bsl[:, 0:L], bsl[:, L:2 * L])
                    # y = y_g0 + y_g1 + u*D ; then gate by silu(z)
                    nc.vector.tensor_add(ygate[m][:], ygate[m][:], bsl[:, 0:L])
                    nc.vector.scalar_tensor_tensor(
                        ygate[m][:], u_silu[half + m][:], D_t[:, m:m + 1], ygate[m][:],
                        OP.mult, OP.add)
                    nc.vector.tensor_mul(ygate[m][:], ygate[m][:], z_silu[m][:])
        bcp.release()
        scanp.release()

        # ---------- out_proj ----------
        wop = tc.alloc_tile_pool(name="wop", bufs=4)
        opool = tc.alloc_tile_pool(name="opool", bufs=2)
        for m in range(NKM):
            ps = ppool.tile((P, L), FP32, name="pso", tag="mm")
            for k in range(NMH):
                wt = wop.tile((P, P), BF16, name="wto", tag="wto", bufs=8)
                nc.sync.dma_start(out=wt[:], in_=woT[k * P:(k + 1) * P, m * P:(m + 1) * P])
                for c in range(2):
                    nc.tensor.matmul(ps[:, c * 512:(c + 1) * 512], wt[:],
                                     ygate[k][:, c * 512:(c + 1) * 512],
                                     start=(k == 0), stop=(k == NMH - 1))
            osb = opool.tile((P, L), FP32, name="osb", tag="osb")
            nc.scalar.copy(osb[:], ps[:])
            nc.sync.dma_start(out=outp[m * P:(m + 1) * P, :], in_=osb[:])
        opool.release()
        wop.release()
        ppool.release()
        main.release()
        const.release()
    if finalize:
        nc.finalize()
    return nc


def _shards(inputs):
    """Build the 8 per-core input maps (numpy, fp32/bf16 via ml_dtypes)."""
    import ml_dtypes

    def bf(a):
        return np.asarray(a, np.float32).astype(ml_dtypes.bfloat16)

    x = np.asarray(inputs["x"], np.float32)
    g = np.asarray(inputs["ln_g"], np.float32)
    be = np.asarray(inputs["ln_b"], np.float32)
    ident = np.eye(P, dtype=np.float32)

    maps = []
    for d, pre in ((0, "f_"), (1, "b_")):
        in_w = np.asarray(inputs[pre + "in_w"], np.float32)
        conv_w = np.asarray(inputs[pre + "conv_w"], np.float32)
        conv_b = np.asarray(inputs[pre + "conv_b"], np.float32)
        xproj_w = np.asarray(inputs[pre + "xproj_w"], np.float32)
        dt_w = np.asarray(inputs[pre + "dt_w"], np.float32)
        dt_b = np.asarray(inputs[pre + "dt_b"], np.float32)
        Alog = np.asarray(inputs[pre + "Alog"], np.float32)
        Dv = np.asarray(inputs[pre + "D"], np.float32)
        out_w = np.asarray(inputs[pre + "out_w"], np.float32)
        A = -np.exp(Alog)  # (DI, DSTATE)

        for b in range(2):
            for h in range(2):
                sl = slice(h * DH, (h + 1) * DH)
                # u channel tiles reordered so this core's half comes first
                order = np.r_[h * DH:(h + 1) * DH, (1 - h) * DH:(2 - h) * DH] if h == 1 else np.arange(DI)
                w_u = in_w[:DI][order] * g[None, :]
                w_z = in_w[DI:][sl] * g[None, :]
                bu_full = (in_w[:DI][order] @ be)
                bz_full = (in_w[DI:][sl] @ be)
                cw = conv_w[order]
                cb = conv_b[order]
                convdiag = np.zeros((DCONV, NMU, P, P), np.float32)
                for k in range(DCONV):
                    for m in range(NMU):
                        np.fill_diagonal(convdiag[k, m], cw[m * P:(m + 1) * P, k])
                Ah = A[sl]  # (DH, 16)
                Amat = Ah.reshape(NMH, P, DSTATE).transpose(1, 0, 2).reshape(P, P)
                xp = xproj_w[:, order]  # (96, DI)
                xs = x[b] if d == 0 else x[b][::-1]
                m = {
                    "xin": np.ascontiguousarray(xs),
                    "wuT": bf(w_u.T),
                    "wzT": bf(w_z.T),
                    "bu": np.ascontiguousarray(bu_full.reshape(NMU, P).T),
                    "bz": np.ascontiguousarray(bz_full.reshape(NMH, P).T),
                    "convd": bf(convdiag),
                    "convb": np.ascontiguousarray(cb.reshape(NMU, P).T),
                    "wxpT": bf(xp.T),
                    "wdtT": bf(dt_w[sl].T),
                    "dtb": np.ascontiguousarray(dt_b[sl].reshape(NMH, P).T),
                    "Amat": np.ascontiguousarray(Amat),
                    "Dvec": np.ascontiguousarray(Dv[sl].reshape(NMH, P).T),
                    "woT": bf(out_w[:, sl].T),
                    "ident": bf(ident),
                }
                maps.append(m)
    return maps


_CACHE = {}


def kernel(**inputs):
    if "nc" not in _CACHE:
        _CACHE["nc"] = build_program()
    nc = _CACHE["nc"]
    maps = _shards(inputs)
    res = run_bass_kernel_spmd(nc, maps, list(range(8)))
    outs = res.results
    x = np.asarray(inputs["x"], np.float32)
    out = x.copy()
    i = 0
    for d in range(2):
        for b in range(2):
            for h in range(2):
                part = outs[i]["outp"].T  # (t, dmo)
                if d == 1:
                    part = part[::-1]
                out[b] += part
                i += 1
    return out


# revision 24
# speedup vs baseline: 1.1627x; 1.0190x over previous
"""BiMamba block Trainium2 kernel.

Sharding: 8 cores = (direction f/b) x (batch 0/1) x (d_inner half 0/1).
Each core is fully independent (no collectives): it computes LN(x[b]),
in_proj u (full 2048, needed for x_proj) + its z half, depthwise causal
conv via PE diag-matmuls, silu, x_proj -> (dt|B|C), dt half, the
selective scan over its 1024 channels (d on partitions, 16 state
segments x time in the free dim, one tensor_tensor_scan per 8-state
slab), gating, and its out_proj column block.  Host flips the sequence
for backward cores and sums the 8 partial outputs + residual.
"""

import sys

sys.path.insert(0, "/opt/trn_rl_repo")

import numpy as np

import concourse.bass as bass
import concourse.mybir as mybir
from concourse import bacc
from concourse.tile import TileContext
from concourse.bass_utils import run_bass_kernel_spmd

FP32 = mybir.dt.float32
BF16 = mybir.dt.bfloat16
AX = mybir.AxisListType
OP = mybir.AluOpType
AF = mybir.ActivationFunctionType

P = 128
L = 1024          # sequence length
DM = 1024         # d_model
DI = 2048         # d_inner
DH = 1024         # d_inner half per core
DSTATE = 16
DTRANK = 64
DCONV = 4
NT = L // P       # 8 t-tiles
NKM = DM // P     # 8 d_model tiles
NMU = DI // P     # 16 u M-tiles
NMH = DH // P     # 8 half M-tiles
GSEG = 8          # states per scan slab
SLABF = GSEG * L  # slab free size


def build_program(finalize=True):
    nc = bacc.Bacc("TRN2", target_bir_lowering=False, debug=False)

    # ---- DRAM I/O (per-core shards; same names on every core) ----
    xin = nc.dram_tensor("xin", (L, DM), FP32, kind="ExternalInput")
    wuT = nc.dram_tensor("wuT", (NMU, P, NKM * P), BF16, kind="ExternalInput")
    wzT = nc.dram_tensor("wzT", (NMH, P, NKM * P), BF16, kind="ExternalInput")
    bu = nc.dram_tensor("bu", (P, NMU), FP32, kind="ExternalInput")
    bz = nc.dram_tensor("bz", (P, NMH), FP32, kind="ExternalInput")
    convd = nc.dram_tensor("convd", (NMU, P, DCONV * P), BF16, kind="ExternalInput")
    convb = nc.dram_tensor("convb", (P, NMU), FP32, kind="ExternalInput")
    wxpT = nc.dram_tensor("wxpT", (P, NMU * 96), BF16, kind="ExternalInput")
    wdtT = nc.dram_tensor("wdtT", (DTRANK, DH), BF16, kind="ExternalInput")
    dtb = nc.dram_tensor("dtb", (P, NMH), FP32, kind="ExternalInput")
    Amat = nc.dram_tensor("Amat", (P, P), FP32, kind="ExternalInput")
    Dvec = nc.dram_tensor("Dvec", (P, NMH), FP32, kind="ExternalInput")
    woT = nc.dram_tensor("woT", (NKM, P, NMH * P), BF16, kind="ExternalInput")
    ident = nc.dram_tensor("ident", (P, P), BF16, kind="ExternalInput")
    outp = nc.dram_tensor("outp", (DM, L), FP32, kind="ExternalOutput")

    with TileContext(nc) as tc:
        const = tc.alloc_tile_pool(name="const", bufs=1)
        main = tc.alloc_tile_pool(name="main", bufs=1)

        bu_t = const.tile((P, NMU), FP32, name="bu_t")
        bz_t = const.tile((P, NMH), FP32, name="bz_t")
        convb_t = const.tile((P, NMU), FP32, name="convb_t")
        dtb_t = const.tile((P, NMH), FP32, name="dtb_t")
        A_t = const.tile((P, P), FP32, name="A_t")
        D_t = const.tile((P, NMH), FP32, name="D_t")
        id_t = const.tile((P, P), BF16, name="id_t")
        for dst, src in ((bu_t, bu), (bz_t, bz), (convb_t, convb),
                         (dtb_t, dtb), (A_t, Amat), (D_t, Dvec), (id_t, ident)):
            nc.sync.dma_start(out=dst[:], in_=src[:])

        # persistent activations
        u_silu = [main.tile((P, L), BF16, name=f"usl{m}", tag=f"usl{m}") for m in range(NMU)]
        z_silu = [main.tile((P, L), BF16, name=f"zsl{m}", tag=f"zsl{m}") for m in range(NMH)]
        dt_sb = [main.tile((P, L), BF16, name=f"dt{m}", tag=f"dt{m}") for m in range(NMH)]
        ygate = [main.tile((P, L), BF16, name=f"yg{m}", tag=f"yg{m}") for m in range(NMH)]
        dbc_bf = main.tile((96, L), BF16, name="dbc_bf", tag="dbc")

        # ---------- front phase: LN -> transpose -> in_proj -> conv ----------
        front = tc.alloc_tile_pool(name="front", bufs=1)
        wpool = tc.alloc_tile_pool(name="wpool", bufs=16)
        ppool = tc.alloc_tile_pool(name="ppool", bufs=3, space="PSUM")
        trpool = tc.alloc_tile_pool(name="trpool", bufs=2, space="PSUM")

        xnT = [front.tile((P, L), BF16, name=f"xnT{k}", tag=f"xnT{k}") for k in range(NKM)]

        for tt in range(NT):
            xt = front.tile((P, DM), FP32, name="xt", tag="xt", bufs=2)
            nc.sync.dma_start(out=xt[:], in_=xin[tt * P:(tt + 1) * P, :])
            ssum = front.tile((P, 1), FP32, name="ssum", tag="stats", bufs=4)
            xsq = front.tile((P, DM), FP32, name="xsq", tag="xsq", bufs=1)
            nc.scalar.activation(xsq[:], xt[:], AF.Identity, accum_out=ssum[:])
            ssq = front.tile((P, 1), FP32, name="ssq", tag="stats", bufs=4)
            nc.scalar.activation(xsq[:], xt[:], AF.Square, accum_out=ssq[:])
            mu = front.tile((P, 1), FP32, name="mu", tag="stats", bufs=4)
            nc.vector.tensor_scalar_mul(mu[:], ssum[:], 1.0 / DM)
            ex2 = front.tile((P, 1), FP32, name="ex2", tag="stats", bufs=4)
            nc.vector.tensor_scalar_mul(ex2[:], ssq[:], 1.0 / DM)
            mu2 = front.tile((P, 1), FP32, name="mu2", tag="stats", bufs=4)
            nc.vector.tensor_mul(mu2[:], mu[:], mu[:])
            var = front.tile((P, 1), FP32, name="var", tag="stats", bufs=4)
            nc.vector.tensor_tensor(var[:], ex2[:], mu2[:], OP.subtract)
            nc.vector.tensor_scalar_add(var[:], var[:], 1e-5)
            sd = front.tile((P, 1), FP32, name="sd", tag="stats", bufs=4)
            nc.scalar.activation(sd[:], var[:], AF.Sqrt)
            r = front.tile((P, 1), FP32, name="r", tag="stats", bufs=4)
            nc.vector.reciprocal(r[:], sd[:])
            xn = front.tile((P, DM), BF16, name="xn", tag="xn", bufs=2)
            nc.vector.tensor_scalar(xn[:], xt[:], mu[:], r[:], OP.subtract, OP.mult)
            # transpose this t-tile into the 8 xnT tiles
            for kk in range(NKM):
                tr = trpool.tile((P, P), BF16, name="tr", tag="tr")
                nc.tensor.transpose(tr[:], xn[:, kk * P:(kk + 1) * P], id_t[:])
                nc.scalar.copy(xnT[kk][:, tt * P:(tt + 1) * P], tr[:])

        # in_proj: u (16 M-tiles) then z (8 M-tiles)
        u_pre = []
        for m in range(NMU):
            ps = ppool.tile((P, L), FP32, name="ps", tag="mm")
            wt = wpool.tile((P, NKM * P), BF16, name="wt", tag="wt", bufs=2)
            nc.sync.dma_start(out=wt[:], in_=wuT[m])
            for k in range(NKM):
                for c in range(2):
                    nc.tensor.matmul(ps[:, c * 512:(c + 1) * 512],
                                     wt[:, k * P:(k + 1) * P],
                                     xnT[k][:, c * 512:(c + 1) * 512],
                                     start=(k == 0), stop=(k == NKM - 1))
            up = front.tile((P, L + DCONV), BF16, name="up", tag="upre", bufs=2)
            nc.vector.memset(up[:, 0:DCONV], 0.0)
            nc.scalar.activation(up[:, DCONV:], ps[:], AF.Identity, bias=bu_t[:, m:m + 1])
            u_pre.append(up)
            # conv for this m-tile: 4 shifted diag matmuls
            pc = ppool.tile((P, L), FP32, name="pc", tag="mm")
            cw = wpool.tile((P, DCONV * P), BF16, name="cw", tag="cw", bufs=2)
            nc.sync.dma_start(out=cw[:], in_=convd[m])
            for k in range(DCONV):
                for c in range(2):
                    nc.tensor.matmul(pc[:, c * 512:(c + 1) * 512],
                                     cw[:, k * P:(k + 1) * P],
                                     up[:, k + 1 + c * 512:k + 1 + (c + 1) * 512],
                                     start=(k == 0), stop=(k == DCONV - 1))
            sig = front.tile((P, L), BF16, name="sig", tag="sig", bufs=2)
            nc.scalar.activation(sig[:], pc[:], AF.Sigmoid, bias=convb_t[:, m:m + 1])
            ucb = front.tile((P, L), BF16, name="ucb", tag="ucb", bufs=2)
            nc.vector.tensor_scalar_add(ucb[:], pc[:], convb_t[:, m:m + 1])
            nc.vector.tensor_mul(u_silu[m][:], sig[:], ucb[:])

        # x_proj: dbc (96, L) = wxpT.T @ u_silu
        pxp = ppool.tile((96, L), FP32, name="pxp", tag="mm")
        wtx = wpool.tile((P, NMU * 96), BF16, name="wtx", tag="wtx", bufs=1)
        nc.sync.dma_start(out=wtx[:], in_=wxpT[:])
        for k in range(NMU):
            for c in range(2):
                nc.tensor.matmul(pxp[:, c * 512:(c + 1) * 512],
                                 wtx[:, k * 96:(k + 1) * 96],
                                 u_silu[k][:, c * 512:(c + 1) * 512],
                                 start=(k == 0), stop=(k == NMU - 1))
        nc.scalar.copy(dbc_bf[:], pxp[:])

        # dt half: softplus(wdtT.T @ dbc[:64] + dtb)
        wdt = const.tile((DTRANK, DH), BF16, name="wdt")
        nc.sync.dma_start(out=wdt[:], in_=wdtT[:])
        for m in range(NMH):
            ps = ppool.tile((P, L), FP32, name="psd", tag="mm")
            for c in range(2):
                nc.tensor.matmul(ps[:, c * 512:(c + 1) * 512], wdt[:, m * P:(m + 1) * P],
                                 dbc_bf[0:DTRANK, c * 512:(c + 1) * 512],
                                 start=True, stop=True)
            edt = front.tile((P, L), BF16, name="edt", tag="edt", bufs=2)
            nc.scalar.activation(edt[:], ps[:], AF.Exp, bias=dtb_t[:, m:m + 1])
            nc.scalar.activation(dt_sb[m][:], edt[:], AF.Ln, bias=1.0)

        for m in range(NMH):
            ps = ppool.tile((P, L), FP32, name="ps", tag="mm")
            wt = wpool.tile((P, NKM * P), BF16, name="wt", tag="wt", bufs=2)
            nc.sync.dma_start(out=wt[:], in_=wzT[m])
            for k in range(NKM):
                for c in range(2):
                    nc.tensor.matmul(ps[:, c * 512:(c + 1) * 512],
                                     wt[:, k * P:(k + 1) * P],
                                     xnT[k][:, c * 512:(c + 1) * 512],
                                     start=(k == 0), stop=(k == NKM - 1))
            sigz = front.tile((P, L), BF16, name="sigz", tag="sig", bufs=2)
            nc.scalar.activation(sigz[:], ps[:], AF.Sigmoid, bias=bz_t[:, m:m + 1])
            zcb = front.tile((P, L), BF16, name="zcb", tag="ucb", bufs=2)
            nc.scalar.activation(zcb[:], ps[:], AF.Identity, bias=bz_t[:, m:m + 1])
            nc.vector.tensor_mul(z_silu[m][:], sigz[:], zcb[:])


        trpool.release()
        wpool.release()
        front.release()

        # ---------- scan phase ----------
        scanp = tc.alloc_tile_pool(name="scanp", bufs=5)
        bcp = tc.alloc_tile_pool(name="bcp", bufs=2)
        half = 0  # u_silu index offset is chosen on host via weight sharding
        # NOTE: the dt/z/out shards shipped by the host correspond to
        # u_silu tiles [hoff .. hoff+8); hoff is encoded by shipping the
        # matching halves of conv/dt/out weights, with u channel tiles
        # relabeled so that tiles 0..7 of the half come first.  The host
        # arranges wuT/convd so that THIS core's half occupies m=0..7.
        ones_row = const.tile((1, P), BF16, name="ones_row")
        nc.vector.memset(ones_row[:], 1.0)

        for g in range(2):
            Bsl = bcp.tile((P, SLABF), BF16, name="Bsl", tag="bc")
            Csl = bcp.tile((P, SLABF), BF16, name="Csl", tag="bc")
            for j in range(GSEG):
                n = g * GSEG + j
                for dst, row in ((Bsl, DTRANK + n), (Csl, DTRANK + DSTATE + n)):
                    stage = scanp.tile((1, L), BF16, name="stage", tag="stage", bufs=2)
                    nc.sync.dma_start(out=stage[:], in_=dbc_bf[row:row + 1, :])
                    pb = ppool.tile((P, L), FP32, name="pb", tag="mm")
                    for c in range(2):
                        nc.tensor.matmul(pb[:, c * 512:(c + 1) * 512], ones_row[:],
                                         stage[:, c * 512:(c + 1) * 512],
                                         start=True, stop=True)
                    nc.scalar.copy(dst[:, j * L:(j + 1) * L], pb[:])
            for m in range(NMH):
                dtu = scanp.tile((P, L), BF16, name="dtu", tag="dtu", bufs=2)
                nc.vector.tensor_mul(dtu[:], dt_sb[m][:], u_silu[half + m][:])
                asl = scanp.tile((P, SLABF), BF16, name="asl", tag="slab")
                for j in range(GSEG):
                    n = g * GSEG + j
                    nc.scalar.activation(asl[:, j * L:(j + 1) * L], dt_sb[m][:],
                                         AF.Exp, scale=A_t[:, m * DSTATE + n:m * DSTATE + n + 1])
                nc.vector.memset(asl[:, 0:SLABF:L], 0.0)
                bsl = scanp.tile((P, SLABF), BF16, name="bsl", tag="slab")
                for j in range(GSEG):
                    nc.vector.tensor_mul(bsl[:, j * L:(j + 1) * L], dtu[:],
                                         Bsl[:, j * L:(j + 1) * L])
                hsl = scanp.tile((P, SLABF), BF16, name="hsl", tag="slab")
                nc.vector.tensor_tensor_scan(hsl[:], asl[:], bsl[:], 0.0, OP.mult, OP.add)
                # ch = h*C into bsl (dead); tree reduce ping-pongs bsl/asl
                nc.vector.tensor_mul(bsl[:], hsl[:], Csl[:])
                nc.vector.tensor_add(asl[:, 0:4 * L], bsl[:, 0:4 * L], bsl[:, 4 * L:8 * L])
                nc.vector.tensor_add(bsl[:, 0:2 * L], asl[:, 0:2 * L], asl[:, 2 * L:4 * L])
                if g == 0:
                    nc.vector.tensor_add(ygate[m][:], bsl[:, 0:L], bsl[:, L:2 * L])
                else:
                    nc.vector.tensor_add(bsl[:, 0:L], bsl[:, 0:L], bsl[:, L:2 * L])
                    # y = y_g0 + y_g1 + u*D ; then gate by silu(z)
                    nc.vector.tensor_add(ygate[m][:], ygate[m][:], bsl[:, 0:L])
                    nc.vector.scalar_tensor_tensor(
                        ygate[m][:], u_silu[half + m][:], D_t[:, m:m + 1], ygate[m][:],
                        OP.mult, OP.add)
                    nc.vector.tensor_mul(ygate[m][:], ygate[m][:], z_silu[m][:])
        bcp.release()
        scanp.release()

        # ---------- out_proj ----------
        wop = tc.alloc_tile_pool(name="wop", bufs=4)
        opool = tc.alloc_tile_pool(name="opool", bufs=2)
        for m in range(NKM):
            ps = ppool.tile((P, L), FP32, name="pso", tag="mm")
            wt = wop.tile((P, NMH * P), BF16, name="wto", tag="wto", bufs=2)
            nc.sync.dma_start(out=wt[:], in_=woT[m])
            for k in range(NMH):
                for c in range(2):
                    nc.tensor.matmul(ps[:, c * 512:(c + 1) * 512],
                                     wt[:, k * P:(k + 1) * P],
                                     ygate[k][:, c * 512:(c + 1) * 512],
                                     start=(k == 0), stop=(k == NMH - 1))
            osb = opool.tile((P, L), FP32, name="osb", tag="osb")
            nc.scalar.copy(osb[:], ps[:])
            nc.sync.dma_start(out=outp[m * P:(m + 1) * P, :], in_=osb[:])
        opool.release()
        wop.release()
        ppool.release()
        main.release()
        const.release()
    if finalize:
        nc.finalize()
    return nc


def _shards(inputs):
    """Build the 8 per-core input maps (numpy, fp32/bf16 via ml_dtypes)."""
    import ml_dtypes

    def bf(a):
        return np.asarray(a, np.float32).astype(ml_dtypes.bfloat16)

    x = np.asarray(inputs["x"], np.float32)
    g = np.asarray(inputs["ln_g"], np.float32)
    be = np.asarray(inputs["ln_b"], np.float32)
    ident = np.eye(P, dtype=np.float32)

    maps = []
    for d, pre in ((0, "f_"), (1, "b_")):
        in_w = np.asarray(inputs[pre + "in_w"], np.float32)
        conv_w = np.asarray(inputs[pre + "conv_w"], np.float32)
        conv_b = np.asarray(inputs[pre + "conv_b"], np.float32)
        xproj_w = np.asarray(inputs[pre + "xproj_w"], np.float32)
        dt_w = np.asarray(inputs[pre + "dt_w"], np.float32)
        dt_b = np.asarray(inputs[pre + "dt_b"], np.float32)
        Alog = np.asarray(inputs[pre + "Alog"], np.float32)
        Dv = np.asarray(inputs[pre + "D"], np.float32)
        out_w = np.asarray(inputs[pre + "out_w"], np.float32)
        A = -np.exp(Alog)  # (DI, DSTATE)

        for b in range(2):
            for h in range(2):
                sl = slice(h * DH, (h + 1) * DH)
                # u channel tiles reordered so this core's half comes first
                order = np.r_[h * DH:(h + 1) * DH, (1 - h) * DH:(2 - h) * DH] if h == 1 else np.arange(DI)
                w_u = in_w[:DI][order] * g[None, :]
                w_z = in_w[DI:][sl] * g[None, :]
                bu_full = (in_w[:DI][order] @ be)
                bz_full = (in_w[DI:][sl] @ be)
                cw = conv_w[order]
                cb = conv_b[order]
                convdiag = np.zeros((NMU, P, DCONV, P), np.float32)
                for k in range(DCONV):
                    for m in range(NMU):
                        np.fill_diagonal(convdiag[m, :, k, :], cw[m * P:(m + 1) * P, k])
                convdiag = convdiag.reshape(NMU, P, DCONV * P)
                Ah = A[sl]  # (DH, 16)
                Amat = Ah.reshape(NMH, P, DSTATE).transpose(1, 0, 2).reshape(P, P)
                xp = xproj_w[:, order]  # (96, DI)
                xs = x[b] if d == 0 else x[b][::-1]
                m = {
                    "xin": np.ascontiguousarray(xs),
                    "wuT": bf(np.ascontiguousarray(
                        w_u.T.reshape(NKM, P, NMU, P).transpose(2, 1, 0, 3)
                        .reshape(NMU, P, NKM * P))),
                    "wzT": bf(np.ascontiguousarray(
                        w_z.T.reshape(NKM, P, NMH, P).transpose(2, 1, 0, 3)
                        .reshape(NMH, P, NKM * P))),
                    "bu": np.ascontiguousarray(bu_full.reshape(NMU, P).T),
                    "bz": np.ascontiguousarray(bz_full.reshape(NMH, P).T),
                    "convd": bf(convdiag),
                    "convb": np.ascontiguousarray(cb.reshape(NMU, P).T),
                    "wxpT": bf(np.ascontiguousarray(
                        xp.T.reshape(NMU, P, 96).transpose(1, 0, 2)
                        .reshape(P, NMU * 96))),
                    "wdtT": bf(dt_w[sl].T),
                    "dtb": np.ascontiguousarray(dt_b[sl].reshape(NMH, P).T),
                    "Amat": np.ascontiguousarray(Amat),
                    "Dvec": np.ascontiguousarray(Dv[sl].reshape(NMH, P).T),
                    "woT": bf(np.ascontiguousarray(
                        out_w[:, sl].T.reshape(NMH, P, NKM, P).transpose(2, 1, 0, 3)
                        .reshape(NKM, P, NMH * P))),
                    "ident": bf(ident),
                }
                maps.append(m)
    return maps


_CACHE = {}


def kernel(**inputs):
    if "nc" not in _CACHE:
        _CACHE["nc"] = build_program()
    nc = _CACHE["nc"]
    maps = _shards(inputs)
    res = run_bass_kernel_spmd(nc, maps, list(range(8)))
    outs = res.results
    x = np.asarray(inputs["x"], np.float32)
    out = x.copy()
    i = 0
    for d in range(2):
        for b in range(2):
            for h in range(2):
                part = outs[i]["outp"].T  # (t, dmo)
                if d == 1:
                    part = part[::-1]
                out[b] += part
                i += 1
    return out
